# revision 12
# baseline (speedup 1.0000x reference)
"""Chamfer distance (dist1 mean only) on 8 trn2 NeuronCores.

Sharding: data-parallel over batch B=8, one batch per core. Each core
computes sum_i min_j ||x_i - y_j||^2 / 65536 for its batch; host sums the
8 partial scalars.

Per-core algorithm:
  min_j d(i,j) = x2_i - 2 * max_j (x_i . y_j - 0.5*y2_j)
The inner term is a K=4 matmul: lhsT rows = (x0, x1, x2, -0.5),
rhs rows = (y0, y1, y2, y2), spread over the four PE row groups
(tile_position).  The max-reduction over j runs on VectorE as
tensor_scalar ops with a max accum_out, reading PSUM directly (the only
fast DVE path measured on this part); per-chunk partial maxes land in
M_cols and are combined with one small reduce at the end.
"""

from contextlib import ExitStack

import numpy as np

import concourse.bass as bass
import concourse.tile as tile
from concourse import bacc
from concourse import mybir
from concourse.bass_utils import run_bass_kernel_spmd

F32 = mybir.dt.float32

B = 8
PTS = 8192            # points per batch (both clouds)
P = 128               # i-chunk size (PSUM partitions)
JTILE = 512           # matmul free dim (one PSUM bank)
SUPER = 2048          # superblock free dim (4 banks)
QUADS = PTS // SUPER  # 4 superblocks per i-chunk
NEG_INIT = -3.0e38
SCALE = 1.0 / (B * PTS)  # each core contributes sum/65536


def build(n_chunks=PTS // P):
    nc = bacc.Bacc(None)
    xT = nc.declare_dram_parameter("xT", [4, PTS], F32, isOutput=False)
    yT = nc.declare_dram_parameter("yT", [4, PTS], F32, isOutput=False)
    y64 = nc.declare_dram_parameter("y64", [64, 384], F32, isOutput=False)
    x128 = nc.declare_dram_parameter("x128", [128, 192], F32, isOutput=False)
    out = nc.declare_dram_parameter("out", [1, 1], F32, isOutput=True)

    with ExitStack() as ctx:
        tc = ctx.enter_context(tile.TileContext(nc))
        singles = ctx.enter_context(tc.tile_pool(name="singles", bufs=1))
        ps_pool = ctx.enter_context(tc.tile_pool(name="ps", bufs=2, space="PSUM"))

        lhsT_sb = singles.tile([128, PTS], F32)
        rhs_sb = singles.tile([128, PTS], F32)
        scr = singles.tile([128, SUPER], F32)
        M_cols = singles.tile([128, QUADS * n_chunks], F32)
        M_nat = singles.tile([128, n_chunks], F32)

        # x / y data replicated into the four 32-partition row groups.
        # yT row 3 is overwritten by y2 (computed on device) before use.
        for r in range(4):
            nc.sync.dma_start(out=lhsT_sb[32 * r : 32 * r + 4, :], in_=xT[:])
            nc.sync.dma_start(out=rhs_sb[32 * r : 32 * r + 3, :], in_=yT[0:3, :])

        # y2[j] = |y_j|^2 computed in [64,128] layout, then flattened into the
        # j-ordered row (j = c*128 + p ordering matches yT columns).
        y64_sb = singles.tile([64, 384], F32)
        nc.scalar.dma_start(out=y64_sb, in_=y64[:])
        sq_y = singles.tile([64, 384], F32)
        nc.vector.tensor_mul(sq_y, y64_sb, y64_sb)
        sq_y3 = sq_y.rearrange("p (q d) -> p d q", d=3)
        tmp_y = singles.tile([64, 128], F32)
        nc.vector.tensor_add(tmp_y, sq_y3[:, 0, :], sq_y3[:, 1, :])
        y2t = singles.tile([64, 128], F32)
        nc.vector.tensor_add(y2t, tmp_y, sq_y3[:, 2, :])

        # x2[i] = |x_i|^2 in [128, n_chunks] layout (i = c*128 + p).
        x128_sb = singles.tile([128, 192], F32)
        nc.scalar.dma_start(out=x128_sb, in_=x128[:])
        sq_x = singles.tile([128, 192], F32)
        nc.vector.tensor_mul(sq_x, x128_sb, x128_sb)
        sq_x3 = sq_x.rearrange("p (q d) -> p d q", d=3)
        tmp_x = singles.tile([128, 64], F32)
        nc.vector.tensor_add(tmp_x, sq_x3[:, 0, :], sq_x3[:, 1, :])
        x2_nat = singles.tile([128, 64], F32)
        nc.vector.tensor_add(x2_nat, tmp_x, sq_x3[:, 2, :])

        # Barrier before the y2-row scatter: collapses its dependencies
        # (y2t compute + the yT loads into rhs_sb) into a single wait so the
        # DMA doesn't exceed the per-instruction sync-wait limit.
        tc.strict_bb_all_engine_barrier()

        # Partition-crossing gather [64,128] -> [1,8192] (j = c*128 + p),
        # then replicate into the other three row groups.
        nc.gpsimd.dma_start(out=rhs_sb[3:4, :], in_=y2t[:, :])
        for r in range(1, 4):
            nc.gpsimd.dma_start(
                out=rhs_sb[32 * r + 3 : 32 * r + 4, :], in_=rhs_sb[3:4, :]
            )

        # Collapse the many setup-DMA semaphores into one wait point so the
        # first matmuls don't exceed the per-instruction sync-wait limit.
        tc.strict_bb_all_engine_barrier()

        for c in range(n_chunks):
            for q in range(QUADS):
                ps = ps_pool.tile([128, SUPER], F32, tag="ps")
                for r in range(4):
                    j0 = (q * 4 + r) * JTILE
                    nc.tensor.matmul(
                        out=ps[:, r * JTILE : (r + 1) * JTILE],
                        lhsT=lhsT_sb[32 * r : 32 * r + 4, c * P : (c + 1) * P],
                        rhs=rhs_sb[32 * r : 32 * r + 4, j0 : j0 + JTILE],
                        start=True,
                        stop=True,
                        tile_position=(32 * r, 0),
                    )
                # max over this superblock straight out of PSUM (1x path)
                nc.vector.tensor_scalar(
                    out=scr,
                    in0=ps,
                    scalar1=1.0,
                    scalar2=None,
                    op0=mybir.AluOpType.mult,
                    op1=mybir.AluOpType.max,
                    accum_out=M_cols[:, c * QUADS + q : c * QUADS + q + 1],
                )

        # combine the per-superblock maxes: [128, (c q)] -> [128, c]
        nc.vector.tensor_reduce(
            out=M_nat,
            in_=M_cols.rearrange("p (c q) -> p c q", q=QUADS),
            axis=mybir.AxisListType.X,
            op=mybir.AluOpType.max,
        )

        # partial = sum_i (x2_i - 2*M_i) * SCALE ; then partition-sum via PE.
        M2 = singles.tile([128, n_chunks], F32)
        nc.vector.tensor_scalar_mul(M2, M_nat, -2.0)
        E_sum = singles.tile([128, n_chunks], F32)
        nc.vector.tensor_add(E_sum, x2_nat[:, 0:n_chunks], M2)
        part = singles.tile([128, 1], F32)
        nc.vector.tensor_scalar(
            out=scr[:, 0:n_chunks],
            in0=E_sum,
            scalar1=SCALE,
            scalar2=None,
            op0=mybir.AluOpType.mult,
            op1=mybir.AluOpType.add,
            accum_out=part,
        )
        ones_col = singles.tile([128, 1], F32)
        nc.vector.memset(ones_col, 1.0)
        ps_fin = ps_pool.tile([1, 1], F32, tag="ps")
        nc.tensor.matmul(
            out=ps_fin, lhsT=part, rhs=ones_col, start=True, stop=True
        )
        out_sb = singles.tile([1, 1], F32)
        nc.scalar.copy(out=out_sb, in_=ps_fin)
        nc.sync.dma_start(out=out[:], in_=out_sb)

    nc.compile()
    if not nc.is_finalized():
        nc.finalize()
    return nc


def make_in_maps(xyz1, xyz2):
    in_maps = []
    for b in range(B):
        x = np.ascontiguousarray(xyz1[b], dtype=np.float32)  # [8192, 3]
        y = np.ascontiguousarray(xyz2[b], dtype=np.float32)
        xT = np.empty((4, PTS), dtype=np.float32)
        xT[0:3] = x.T
        xT[3] = -0.5
        yT = np.empty((4, PTS), dtype=np.float32)
        yT[0:3] = y.T
        yT[3] = 0.0  # overwritten on device by y2
        y64 = np.ascontiguousarray(y.reshape(64, 384))
        x128 = np.ascontiguousarray(
            x.reshape(64, 128, 3).transpose(1, 0, 2).reshape(128, 192)
        )
        in_maps.append({"xT": xT, "yT": yT, "y64": y64, "x128": x128})
    return in_maps


def _run(xyz1, xyz2, trace=False):
    nc = build()
    in_maps = make_in_maps(xyz1, xyz2)
    res = run_bass_kernel_spmd(nc, in_maps, list(range(B)), trace=trace)
    total = np.float64(0.0)
    for r in res.results:
        total += np.float64(r["out"][0, 0])
    return np.asarray(total, dtype=np.float32), res


def kernel(xyz1, xyz2):
    out, _ = _run(np.asarray(xyz1), np.asarray(xyz2), trace=False)
    return out


# revision 13
# speedup vs baseline: 1.0008x; 1.0008x over previous
"""Chamfer distance (dist1 mean only) on 8 trn2 NeuronCores.

Sharding: data-parallel over batch B=8, one batch per core. Each core
computes sum_i min_j ||x_i - y_j||^2 / 65536 for its batch; host sums the
8 partial scalars.

Per-core algorithm:
  min_j d(i,j) = x2_i - 2 * max_j (x_i . y_j - 0.5*y2_j)
The inner term is a K=4 matmul: lhsT rows = (x0, x1, x2, -0.5),
rhs rows = (y0, y1, y2, y2), spread over the four PE row groups
(tile_position).  The max-reduction over j runs on VectorE as
tensor_scalar ops with a max accum_out, reading PSUM directly (the only
fast DVE path measured on this part); per-chunk partial maxes land in
M_cols and are combined with one small reduce at the end.
"""

from contextlib import ExitStack

import numpy as np

import concourse.bass as bass
import concourse.tile as tile
from concourse import bacc
from concourse import mybir
from concourse.bass_utils import run_bass_kernel_spmd

F32 = mybir.dt.float32

B = 8
PTS = 8192            # points per batch (both clouds)
P = 128               # i-chunk size (PSUM partitions)
JTILE = 512           # matmul free dim (one PSUM bank)
SUPER = 2048          # superblock free dim (4 banks)
QUADS = PTS // SUPER  # 4 superblocks per i-chunk
NEG_INIT = -3.0e38
SCALE = 1.0 / (B * PTS)  # each core contributes sum/65536


def build(n_chunks=PTS // P):
    nc = bacc.Bacc(None)
    xT = nc.declare_dram_parameter("xT", [4, PTS], F32, isOutput=False)
    yT = nc.declare_dram_parameter("yT", [4, PTS], F32, isOutput=False)
    y64 = nc.declare_dram_parameter("y64", [64, 384], F32, isOutput=False)
    x128 = nc.declare_dram_parameter("x128", [128, 192], F32, isOutput=False)
    out = nc.declare_dram_parameter("out", [1, 1], F32, isOutput=True)

    with ExitStack() as ctx:
        tc = ctx.enter_context(tile.TileContext(nc))
        singles = ctx.enter_context(tc.tile_pool(name="singles", bufs=1))
        ps_pool = ctx.enter_context(tc.tile_pool(name="ps", bufs=2, space="PSUM"))

        lhsT_sb = singles.tile([128, PTS], F32)
        rhs_sb = singles.tile([128, PTS], F32)
        scr = singles.tile([128, SUPER], F32)
        M_cols = singles.tile([128, QUADS * n_chunks], F32)
        M_nat = singles.tile([128, n_chunks], F32)

        # x / y data replicated into the four 32-partition row groups.
        # yT row 3 is overwritten by y2 (computed on device) before use.
        for r in range(4):
            nc.sync.dma_start(out=lhsT_sb[32 * r : 32 * r + 4, :], in_=xT[:])
            nc.sync.dma_start(out=rhs_sb[32 * r : 32 * r + 3, :], in_=yT[0:3, :])

        # y2[j] = |y_j|^2 computed in [64,128] layout, then flattened into the
        # j-ordered row (j = c*128 + p ordering matches yT columns).
        y64_sb = singles.tile([64, 384], F32)
        nc.scalar.dma_start(out=y64_sb, in_=y64[:])
        sq_y = singles.tile([64, 384], F32)
        nc.vector.tensor_mul(sq_y, y64_sb, y64_sb)
        sq_y3 = sq_y.rearrange("p (q d) -> p d q", d=3)
        tmp_y = singles.tile([64, 128], F32)
        nc.vector.tensor_add(tmp_y, sq_y3[:, 0, :], sq_y3[:, 1, :])
        y2t = singles.tile([64, 128], F32)
        nc.vector.tensor_add(y2t, tmp_y, sq_y3[:, 2, :])

        # x2[i] = |x_i|^2 in [128, n_chunks] layout (i = c*128 + p).
        x128_sb = singles.tile([128, 192], F32)
        nc.scalar.dma_start(out=x128_sb, in_=x128[:])
        sq_x = singles.tile([128, 192], F32)
        nc.vector.tensor_mul(sq_x, x128_sb, x128_sb)
        sq_x3 = sq_x.rearrange("p (q d) -> p d q", d=3)
        tmp_x = singles.tile([128, 64], F32)
        nc.vector.tensor_add(tmp_x, sq_x3[:, 0, :], sq_x3[:, 1, :])
        x2_nat = singles.tile([128, 64], F32)
        nc.vector.tensor_add(x2_nat, tmp_x, sq_x3[:, 2, :])

        # Barrier before the y2-row scatter: collapses its dependencies
        # (y2t compute + the yT loads into rhs_sb) into a single wait so the
        # DMA doesn't exceed the per-instruction sync-wait limit.
        tc.strict_bb_all_engine_barrier()

        # Partition-crossing gather [64,128] -> [1,8192] (j = c*128 + p),
        # then replicate into the other three row groups.
        nc.sync.dma_start(out=rhs_sb[3:4, :], in_=y2t[:, :])
        for r in range(1, 4):
            nc.sync.dma_start(
                out=rhs_sb[32 * r + 3 : 32 * r + 4, :], in_=rhs_sb[3:4, :]
            )

        # Collapse the many setup-DMA semaphores into one wait point so the
        # first matmuls don't exceed the per-instruction sync-wait limit.
        tc.strict_bb_all_engine_barrier()

        for c in range(n_chunks):
            for q in range(QUADS):
                ps = ps_pool.tile([128, SUPER], F32, tag="ps")
                for r in range(4):
                    j0 = (q * 4 + r) * JTILE
                    nc.tensor.matmul(
                        out=ps[:, r * JTILE : (r + 1) * JTILE],
                        lhsT=lhsT_sb[32 * r : 32 * r + 4, c * P : (c + 1) * P],
                        rhs=rhs_sb[32 * r : 32 * r + 4, j0 : j0 + JTILE],
                        start=True,
                        stop=True,
                        tile_position=(32 * r, 0),
                    )
                # max over this superblock straight out of PSUM (1x path)
                nc.vector.tensor_scalar(
                    out=scr,
                    in0=ps,
                    scalar1=1.0,
                    scalar2=None,
                    op0=mybir.AluOpType.mult,
                    op1=mybir.AluOpType.max,
                    accum_out=M_cols[:, c * QUADS + q : c * QUADS + q + 1],
                )

        # combine the per-superblock maxes: [128, (c q)] -> [128, c]
        nc.vector.tensor_reduce(
            out=M_nat,
            in_=M_cols.rearrange("p (c q) -> p c q", q=QUADS),
            axis=mybir.AxisListType.X,
            op=mybir.AluOpType.max,
        )

        # partial = sum_i (x2_i - 2*M_i) * SCALE ; then partition-sum via PE.
        M2 = singles.tile([128, n_chunks], F32)
        nc.vector.tensor_scalar_mul(M2, M_nat, -2.0)
        E_sum = singles.tile([128, n_chunks], F32)
        nc.vector.tensor_add(E_sum, x2_nat[:, 0:n_chunks], M2)
        part = singles.tile([128, 1], F32)
        nc.vector.tensor_scalar(
            out=scr[:, 0:n_chunks],
            in0=E_sum,
            scalar1=SCALE,
            scalar2=None,
            op0=mybir.AluOpType.mult,
            op1=mybir.AluOpType.add,
            accum_out=part,
        )
        ones_col = singles.tile([128, 1], F32)
        nc.vector.memset(ones_col, 1.0)
        ps_fin = ps_pool.tile([1, 1], F32, tag="ps")
        nc.tensor.matmul(
            out=ps_fin, lhsT=part, rhs=ones_col, start=True, stop=True
        )
        out_sb = singles.tile([1, 1], F32)
        nc.scalar.copy(out=out_sb, in_=ps_fin)
        nc.sync.dma_start(out=out[:], in_=out_sb)

    nc.compile()
    if not nc.is_finalized():
        nc.finalize()
    return nc


def make_in_maps(xyz1, xyz2):
    in_maps = []
    for b in range(B):
        x = np.ascontiguousarray(xyz1[b], dtype=np.float32)  # [8192, 3]
        y = np.ascontiguousarray(xyz2[b], dtype=np.float32)
        xT = np.empty((4, PTS), dtype=np.float32)
        xT[0:3] = x.T
        xT[3] = -0.5
        yT = np.empty((4, PTS), dtype=np.float32)
        yT[0:3] = y.T
        yT[3] = 0.0  # overwritten on device by y2
        y64 = np.ascontiguousarray(y.reshape(64, 384))
        x128 = np.ascontiguousarray(
            x.reshape(64, 128, 3).transpose(1, 0, 2).reshape(128, 192)
        )
        in_maps.append({"xT": xT, "yT": yT, "y64": y64, "x128": x128})
    return in_maps


def _run(xyz1, xyz2, trace=False):
    nc = build()
    in_maps = make_in_maps(xyz1, xyz2)
    res = run_bass_kernel_spmd(nc, in_maps, list(range(B)), trace=trace)
    total = np.float64(0.0)
    for r in res.results:
        total += np.float64(r["out"][0, 0])
    return np.asarray(total, dtype=np.float32), res


def kernel(xyz1, xyz2):
    out, _ = _run(np.asarray(xyz1), np.asarray(xyz2), trace=False)
    return out


# revision 15
# speedup vs baseline: 1.0241x; 1.0232x over previous
"""Chamfer distance (dist1 mean only) on 8 trn2 NeuronCores.

Sharding: data-parallel over batch B=8, one batch per core. Each core
computes sum_i min_j ||x_i - y_j||^2 / 65536 for its batch; host sums the
8 partial scalars.

Per-core algorithm:
  min_j d(i,j) = x2_i - 2 * max_j (x_i . y_j - 0.5*y2_j)
The inner term is a K=4 matmul: lhsT rows = (x0, x1, x2, -0.5),
rhs rows = (y0, y1, y2, y2), spread over the four PE row groups
(tile_position).  The max-reduction over j runs on VectorE as
tensor_scalar ops with a max accum_out, reading PSUM directly (the only
fast DVE path measured on this part); per-chunk partial maxes land in
M_cols and are combined with one small reduce at the end.
"""

from contextlib import ExitStack

import numpy as np

import concourse.bass as bass
import concourse.tile as tile
from concourse import bacc
from concourse import mybir
from concourse.bass_utils import run_bass_kernel_spmd

F32 = mybir.dt.float32

B = 8
PTS = 8192            # points per batch (both clouds)
P = 128               # i-chunk size (PSUM partitions)
JTILE = 512           # matmul free dim (one PSUM bank)
SUPER = 2048          # superblock free dim (4 banks)
QUADS = PTS // SUPER  # 4 superblocks per i-chunk
NEG_INIT = -3.0e38
SCALE = 1.0 / (B * PTS)  # each core contributes sum/65536


def build(n_chunks=PTS // P):
    nc = bacc.Bacc(None)
    xT = nc.declare_dram_parameter("xT", [4, PTS], F32, isOutput=False)
    yT = nc.declare_dram_parameter("yT", [4, PTS], F32, isOutput=False)
    y64 = nc.declare_dram_parameter("y64", [64, 384], F32, isOutput=False)
    x128 = nc.declare_dram_parameter("x128", [128, 192], F32, isOutput=False)
    out = nc.declare_dram_parameter("out", [1, 1], F32, isOutput=True)

    with ExitStack() as ctx:
        tc = ctx.enter_context(tile.TileContext(nc))
        singles = ctx.enter_context(tc.tile_pool(name="singles", bufs=1))
        ps_pool = ctx.enter_context(tc.tile_pool(name="ps", bufs=2, space="PSUM"))

        lhsT_sb = singles.tile([128, PTS], F32)
        rhs_sb = singles.tile([128, PTS], F32)
        scr = singles.tile([128, SUPER], F32)
        M_cols = singles.tile([128, QUADS * n_chunks], F32)
        M_nat = singles.tile([128, n_chunks], F32)

        # x data replicated into the four 32-partition row groups.  Row
        # group r only ever consumes the contiguous j-range
        # [r*2048, (r+1)*2048) (see the main loop), so its y rows are
        # loaded for that quarter only.
        for r in range(4):
            nc.sync.dma_start(out=lhsT_sb[32 * r : 32 * r + 4, :], in_=xT[:])
            nc.sync.dma_start(
                out=rhs_sb[32 * r : 32 * r + 3, r * 2048 : (r + 1) * 2048],
                in_=yT[0:3, r * 2048 : (r + 1) * 2048],
            )

        # y2[j] = |y_j|^2 computed in [64,128] layout, then flattened into the
        # j-ordered row (j = c*128 + p ordering matches yT columns).
        y64_sb = singles.tile([64, 384], F32)
        nc.scalar.dma_start(out=y64_sb, in_=y64[:])
        sq_y = singles.tile([64, 384], F32)
        nc.vector.tensor_mul(sq_y, y64_sb, y64_sb)
        sq_y3 = sq_y.rearrange("p (q d) -> p d q", d=3)
        tmp_y = singles.tile([64, 128], F32)
        nc.vector.tensor_add(tmp_y, sq_y3[:, 0, :], sq_y3[:, 1, :])
        y2t = singles.tile([64, 128], F32)
        nc.vector.tensor_add(y2t, tmp_y, sq_y3[:, 2, :])

        # x2[i] = |x_i|^2 in [128, n_chunks] layout (i = c*128 + p).
        x128_sb = singles.tile([128, 192], F32)
        nc.scalar.dma_start(out=x128_sb, in_=x128[:])
        sq_x = singles.tile([128, 192], F32)
        nc.vector.tensor_mul(sq_x, x128_sb, x128_sb)
        sq_x3 = sq_x.rearrange("p (q d) -> p d q", d=3)
        tmp_x = singles.tile([128, 64], F32)
        nc.vector.tensor_add(tmp_x, sq_x3[:, 0, :], sq_x3[:, 1, :])
        x2_nat = singles.tile([128, 64], F32)
        nc.vector.tensor_add(x2_nat, tmp_x, sq_x3[:, 2, :])

        # Barrier before the y2-row scatter: collapses its dependencies
        # (y2t compute + the yT loads into rhs_sb) into a single wait so the
        # DMA doesn't exceed the per-instruction sync-wait limit.
        tc.strict_bb_all_engine_barrier()

        # Partition-crossing y2 scatters: group r gets only its quarter of
        # the row (y2t rows 16r..16r+15, j = c*128 + p), so the four DMAs
        # hit four different partitions in parallel.
        for r in range(4):
            nc.sync.dma_start(
                out=rhs_sb[32 * r + 3 : 32 * r + 4, r * 2048 : (r + 1) * 2048],
                in_=y2t[16 * r : 16 * r + 16, :],
            )

        # Collapse the many setup-DMA semaphores into one wait point so the
        # first matmuls don't exceed the per-instruction sync-wait limit.
        tc.strict_bb_all_engine_barrier()

        for c in range(n_chunks):
            for q in range(QUADS):
                ps = ps_pool.tile([128, SUPER], F32, tag="ps")
                for r in range(4):
                    j0 = (r * 4 + q) * JTILE
                    nc.tensor.matmul(
                        out=ps[:, r * JTILE : (r + 1) * JTILE],
                        lhsT=lhsT_sb[32 * r : 32 * r + 4, c * P : (c + 1) * P],
                        rhs=rhs_sb[32 * r : 32 * r + 4, j0 : j0 + JTILE],
                        start=True,
                        stop=True,
                        tile_position=(32 * r, 0),
                    )
                # max over this superblock straight out of PSUM (1x path)
                nc.vector.tensor_scalar(
                    out=scr,
                    in0=ps,
                    scalar1=1.0,
                    scalar2=None,
                    op0=mybir.AluOpType.mult,
                    op1=mybir.AluOpType.max,
                    accum_out=M_cols[:, c * QUADS + q : c * QUADS + q + 1],
                )

        # combine the per-superblock maxes: [128, (c q)] -> [128, c]
        nc.vector.tensor_reduce(
            out=M_nat,
            in_=M_cols.rearrange("p (c q) -> p c q", q=QUADS),
            axis=mybir.AxisListType.X,
            op=mybir.AluOpType.max,
        )

        # partial = sum_i (x2_i - 2*M_i) * SCALE ; then partition-sum via PE.
        M2 = singles.tile([128, n_chunks], F32)
        nc.vector.tensor_scalar_mul(M2, M_nat, -2.0)
        E_sum = singles.tile([128, n_chunks], F32)
        nc.vector.tensor_add(E_sum, x2_nat[:, 0:n_chunks], M2)
        part = singles.tile([128, 1], F32)
        nc.vector.tensor_scalar(
            out=scr[:, 0:n_chunks],
            in0=E_sum,
            scalar1=SCALE,
            scalar2=None,
            op0=mybir.AluOpType.mult,
            op1=mybir.AluOpType.add,
            accum_out=part,
        )
        ones_col = singles.tile([128, 1], F32)
        nc.vector.memset(ones_col, 1.0)
        ps_fin = ps_pool.tile([1, 1], F32, tag="ps")
        nc.tensor.matmul(
            out=ps_fin, lhsT=part, rhs=ones_col, start=True, stop=True
        )
        out_sb = singles.tile([1, 1], F32)
        nc.scalar.copy(out=out_sb, in_=ps_fin)
        nc.sync.dma_start(out=out[:], in_=out_sb)

    nc.compile()
    if not nc.is_finalized():
        nc.finalize()
    return nc


def make_in_maps(xyz1, xyz2):
    in_maps = []
    for b in range(B):
        x = np.ascontiguousarray(xyz1[b], dtype=np.float32)  # [8192, 3]
        y = np.ascontiguousarray(xyz2[b], dtype=np.float32)
        xT = np.empty((4, PTS), dtype=np.float32)
        xT[0:3] = x.T
        xT[3] = -0.5
        yT = np.empty((4, PTS), dtype=np.float32)
        yT[0:3] = y.T
        yT[3] = 0.0  # overwritten on device by y2
        y64 = np.ascontiguousarray(y.reshape(64, 384))
        x128 = np.ascontiguousarray(
            x.reshape(64, 128, 3).transpose(1, 0, 2).reshape(128, 192)
        )
        in_maps.append({"xT": xT, "yT": yT, "y64": y64, "x128": x128})
    return in_maps


def _run(xyz1, xyz2, trace=False):
    nc = build()
    in_maps = make_in_maps(xyz1, xyz2)
    res = run_bass_kernel_spmd(nc, in_maps, list(range(B)), trace=trace)
    total = np.float64(0.0)
    for r in res.results:
        total += np.float64(r["out"][0, 0])
    return np.asarray(total, dtype=np.float32), res


def kernel(xyz1, xyz2):
    out, _ = _run(np.asarray(xyz1), np.asarray(xyz2), trace=False)
    return out


# revision 16
# speedup vs baseline: 1.0732x; 1.0480x over previous
"""Chamfer distance (dist1 mean only) on 8 trn2 NeuronCores.

Sharding: data-parallel over batch B=8, one batch per core. Each core
computes sum_i min_j ||x_i - y_j||^2 / 65536 for its batch; host sums the
8 partial scalars.

Per-core algorithm:
  min_j d(i,j) = x2_i - 2 * max_j (x_i . y_j - 0.5*y2_j)
The inner term is a K=4 matmul: lhsT rows = (x0, x1, x2, -0.5),
rhs rows = (y0, y1, y2, y2), spread over the four PE row groups
(tile_position).  The max-reduction over j runs on VectorE as
tensor_scalar ops with a max accum_out, reading PSUM directly (the only
fast DVE path measured on this part); per-chunk partial maxes land in
M_cols and are combined with one small reduce at the end.
"""

from contextlib import ExitStack

import numpy as np

import concourse.bass as bass
import concourse.tile as tile
from concourse import bacc
from concourse import mybir
from concourse.bass_utils import run_bass_kernel_spmd

F32 = mybir.dt.float32

B = 8
PTS = 8192            # points per batch (both clouds)
P = 128               # i-chunk size (PSUM partitions)
JTILE = 512           # matmul free dim (one PSUM bank)
SUPER = 2048          # superblock free dim (4 banks)
QUADS = PTS // SUPER  # 4 superblocks per i-chunk
NEG_INIT = -3.0e38
SCALE = 1.0 / (B * PTS)  # each core contributes sum/65536


def build(n_chunks=PTS // P):
    nc = bacc.Bacc(None)
    xT = nc.declare_dram_parameter("xT", [4, PTS], F32, isOutput=False)
    yT = nc.declare_dram_parameter("yT", [4, PTS], F32, isOutput=False)
    y64 = nc.declare_dram_parameter("y64", [64, 384], F32, isOutput=False)
    x128 = nc.declare_dram_parameter("x128", [128, 192], F32, isOutput=False)
    out = nc.declare_dram_parameter("out", [1, 1], F32, isOutput=True)

    with ExitStack() as ctx:
        tc = ctx.enter_context(tile.TileContext(nc))
        singles = ctx.enter_context(tc.tile_pool(name="singles", bufs=1))
        ps_pool = ctx.enter_context(tc.tile_pool(name="ps", bufs=2, space="PSUM"))

        lhsT_sb = singles.tile([128, PTS], F32)
        rhs_sb = singles.tile([128, PTS], F32)
        scr = singles.tile([128, SUPER], F32)
        M_cols = singles.tile([128, QUADS * n_chunks], F32)
        M_nat = singles.tile([128, n_chunks], F32)

        # x data replicated into the four 32-partition row groups.  Row
        # group r only ever consumes the contiguous j-range
        # [r*2048, (r+1)*2048) (see the main loop), so its y rows are
        # loaded for that quarter only.
        for r in range(4):
            nc.sync.dma_start(out=lhsT_sb[32 * r : 32 * r + 4, :], in_=xT[:])
            nc.sync.dma_start(
                out=rhs_sb[32 * r : 32 * r + 3, r * 2048 : (r + 1) * 2048],
                in_=yT[0:3, r * 2048 : (r + 1) * 2048],
            )

        # y2[j] = |y_j|^2 computed in [64,128] layout, then flattened into the
        # j-ordered row (j = c*128 + p ordering matches yT columns).
        y64_sb = singles.tile([64, 384], F32)
        nc.scalar.dma_start(out=y64_sb, in_=y64[:])
        sq_y = singles.tile([64, 384], F32)
        nc.vector.tensor_mul(sq_y, y64_sb, y64_sb)
        sq_y3 = sq_y.rearrange("p (q d) -> p d q", d=3)
        tmp_y = singles.tile([64, 128], F32)
        nc.vector.tensor_add(tmp_y, sq_y3[:, 0, :], sq_y3[:, 1, :])
        y2t = singles.tile([64, 128], F32)
        nc.vector.tensor_add(y2t, tmp_y, sq_y3[:, 2, :])

        # x2[i] = |x_i|^2 in [128, n_chunks] layout (i = c*128 + p).
        x128_sb = singles.tile([128, 192], F32)
        nc.scalar.dma_start(out=x128_sb, in_=x128[:])
        sq_x = singles.tile([128, 192], F32)
        nc.vector.tensor_mul(sq_x, x128_sb, x128_sb)
        sq_x3 = sq_x.rearrange("p (q d) -> p d q", d=3)
        tmp_x = singles.tile([128, 64], F32)
        nc.vector.tensor_add(tmp_x, sq_x3[:, 0, :], sq_x3[:, 1, :])
        x2_nat = singles.tile([128, 64], F32)
        nc.vector.tensor_add(x2_nat, tmp_x, sq_x3[:, 2, :])

        # Barrier before the y2-row scatter: collapses its dependencies
        # (y2t compute + the yT loads into rhs_sb) into a single wait so the
        # DMA doesn't exceed the per-instruction sync-wait limit.
        tc.strict_bb_all_engine_barrier()

        # Partition-crossing y2 scatters: group r gets only its quarter of
        # the row (y2t rows 16r..16r+15, j = c*128 + p), so the four DMAs
        # hit four different partitions in parallel.
        for r in range(4):
            nc.sync.dma_start(
                out=rhs_sb[32 * r + 3 : 32 * r + 4, r * 2048 : (r + 1) * 2048],
                in_=y2t[16 * r : 16 * r + 16, :],
            )

        # Collapse the many setup-DMA semaphores into one wait point so the
        # first matmuls don't exceed the per-instruction sync-wait limit.
        tc.strict_bb_all_engine_barrier()

        for c in range(n_chunks):
            for q in range(QUADS):
                ps = ps_pool.tile([128, SUPER], F32, tag="ps")
                for r in range(4):
                    j0 = (r * 4 + q) * JTILE
                    nc.tensor.matmul(
                        out=ps[:, r * JTILE : (r + 1) * JTILE],
                        lhsT=lhsT_sb[32 * r : 32 * r + 4, c * P : (c + 1) * P],
                        rhs=rhs_sb[32 * r : 32 * r + 4, j0 : j0 + JTILE],
                        start=True,
                        stop=True,
                        tile_position=(32 * r, 0),
                    )
                # max over this superblock straight out of PSUM (1x path);
                # plain tensor_reduce avoids the per-op accumulator-readback
                # instruction and the full-width side write of ts+accum.
                nc.vector.tensor_reduce(
                    out=M_cols[:, c * QUADS + q : c * QUADS + q + 1],
                    in_=ps,
                    axis=mybir.AxisListType.X,
                    op=mybir.AluOpType.max,
                )

        # combine the per-superblock maxes: [128, (c q)] -> [128, c]
        nc.vector.tensor_reduce(
            out=M_nat,
            in_=M_cols.rearrange("p (c q) -> p c q", q=QUADS),
            axis=mybir.AxisListType.X,
            op=mybir.AluOpType.max,
        )

        # partial = sum_i (x2_i - 2*M_i) * SCALE ; then partition-sum via PE.
        M2 = singles.tile([128, n_chunks], F32)
        nc.vector.tensor_scalar_mul(M2, M_nat, -2.0)
        E_sum = singles.tile([128, n_chunks], F32)
        nc.vector.tensor_add(E_sum, x2_nat[:, 0:n_chunks], M2)
        part = singles.tile([128, 1], F32)
        nc.vector.tensor_scalar(
            out=scr[:, 0:n_chunks],
            in0=E_sum,
            scalar1=SCALE,
            scalar2=None,
            op0=mybir.AluOpType.mult,
            op1=mybir.AluOpType.add,
            accum_out=part,
        )
        ones_col = singles.tile([128, 1], F32)
        nc.vector.memset(ones_col, 1.0)
        ps_fin = ps_pool.tile([1, 1], F32, tag="ps")
        nc.tensor.matmul(
            out=ps_fin, lhsT=part, rhs=ones_col, start=True, stop=True
        )
        out_sb = singles.tile([1, 1], F32)
        nc.scalar.copy(out=out_sb, in_=ps_fin)
        nc.sync.dma_start(out=out[:], in_=out_sb)

    nc.compile()
    if not nc.is_finalized():
        nc.finalize()
    return nc


def make_in_maps(xyz1, xyz2):
    in_maps = []
    for b in range(B):
        x = np.ascontiguousarray(xyz1[b], dtype=np.float32)  # [8192, 3]
        y = np.ascontiguousarray(xyz2[b], dtype=np.float32)
        xT = np.empty((4, PTS), dtype=np.float32)
        xT[0:3] = x.T
        xT[3] = -0.5
        yT = np.empty((4, PTS), dtype=np.float32)
        yT[0:3] = y.T
        yT[3] = 0.0  # overwritten on device by y2
        y64 = np.ascontiguousarray(y.reshape(64, 384))
        x128 = np.ascontiguousarray(
            x.reshape(64, 128, 3).transpose(1, 0, 2).reshape(128, 192)
        )
        in_maps.append({"xT": xT, "yT": yT, "y64": y64, "x128": x128})
    return in_maps


def _run(xyz1, xyz2, trace=False):
    nc = build()
    in_maps = make_in_maps(xyz1, xyz2)
    res = run_bass_kernel_spmd(nc, in_maps, list(range(B)), trace=trace)
    total = np.float64(0.0)
    for r in res.results:
        total += np.float64(r["out"][0, 0])
    return np.asarray(total, dtype=np.float32), res


def kernel(xyz1, xyz2):
    out, _ = _run(np.asarray(xyz1), np.asarray(xyz2), trace=False)
    return out


# revision 17
# speedup vs baseline: 1.0748x; 1.0015x over previous
"""Chamfer distance (dist1 mean only) on 8 trn2 NeuronCores.

Sharding: data-parallel over batch B=8, one batch per core. Each core
computes sum_i min_j ||x_i - y_j||^2 / 65536 for its batch; host sums the
8 partial scalars.

Per-core algorithm:
  min_j d(i,j) = x2_i - 2 * max_j (x_i . y_j - 0.5*y2_j)
The inner term is a K=4 matmul: lhsT rows = (x0, x1, x2, -0.5),
rhs rows = (y0, y1, y2, y2), spread over the four PE row groups
(tile_position).  The max-reduction over j runs on VectorE as
tensor_scalar ops with a max accum_out, reading PSUM directly (the only
fast DVE path measured on this part); per-chunk partial maxes land in
M_cols and are combined with one small reduce at the end.
"""

from contextlib import ExitStack

import numpy as np

import concourse.bass as bass
import concourse.tile as tile
from concourse import bacc
from concourse import mybir
from concourse.bass_utils import run_bass_kernel_spmd

F32 = mybir.dt.float32

B = 8
PTS = 8192            # points per batch (both clouds)
P = 128               # i-chunk size (PSUM partitions)
JTILE = 512           # matmul free dim (one PSUM bank)
SUPER = 2048          # superblock free dim (4 banks)
QUADS = PTS // SUPER  # 4 superblocks per i-chunk
NEG_INIT = -3.0e38
SCALE = 1.0 / (B * PTS)  # each core contributes sum/65536


def build(n_chunks=PTS // P):
    nc = bacc.Bacc(None)
    xT = nc.declare_dram_parameter("xT", [4, PTS], F32, isOutput=False)
    yT = nc.declare_dram_parameter("yT", [4, PTS], F32, isOutput=False)
    y64 = nc.declare_dram_parameter("y64", [64, 384], F32, isOutput=False)
    x128 = nc.declare_dram_parameter("x128", [128, 192], F32, isOutput=False)
    out = nc.declare_dram_parameter("out", [1, 1], F32, isOutput=True)

    with ExitStack() as ctx:
        tc = ctx.enter_context(tile.TileContext(nc))
        singles = ctx.enter_context(tc.tile_pool(name="singles", bufs=1))
        ps_pool = ctx.enter_context(tc.tile_pool(name="ps", bufs=2, space="PSUM"))

        lhsT_sb = singles.tile([128, PTS], F32)
        rhs_sb = singles.tile([128, PTS], F32)
        scr = singles.tile([128, SUPER], F32)
        M_cols = singles.tile([128, QUADS * n_chunks], F32)
        M_nat = singles.tile([128, n_chunks], F32)

        # x data replicated into the four 32-partition row groups.  Row
        # group r only ever consumes the contiguous j-range
        # [r*2048, (r+1)*2048) (see the main loop), so its y rows are
        # loaded for that quarter only.
        for r in range(4):
            nc.sync.dma_start(out=lhsT_sb[32 * r : 32 * r + 4, :], in_=xT[:])
            nc.sync.dma_start(
                out=rhs_sb[32 * r : 32 * r + 3, r * 2048 : (r + 1) * 2048],
                in_=yT[0:3, r * 2048 : (r + 1) * 2048],
            )

        # y2[j] = |y_j|^2 computed in [64,128] layout, then flattened into the
        # j-ordered row (j = c*128 + p ordering matches yT columns).
        y64_sb = singles.tile([64, 384], F32)
        nc.scalar.dma_start(out=y64_sb, in_=y64[:])
        sq_y = singles.tile([64, 384], F32)
        nc.vector.tensor_mul(sq_y, y64_sb, y64_sb)
        sq_y3 = sq_y.rearrange("p (q d) -> p d q", d=3)
        tmp_y = singles.tile([64, 128], F32)
        nc.vector.tensor_add(tmp_y, sq_y3[:, 0, :], sq_y3[:, 1, :])
        y2t = singles.tile([64, 128], F32)
        nc.vector.tensor_add(y2t, tmp_y, sq_y3[:, 2, :])

        # x2[i] = |x_i|^2 in [128, n_chunks] layout (i = c*128 + p).
        x128_sb = singles.tile([128, 192], F32)
        nc.scalar.dma_start(out=x128_sb, in_=x128[:])
        sq_x = singles.tile([128, 192], F32)
        nc.vector.tensor_mul(sq_x, x128_sb, x128_sb)
        sq_x3 = sq_x.rearrange("p (q d) -> p d q", d=3)
        tmp_x = singles.tile([128, 64], F32)
        nc.vector.tensor_add(tmp_x, sq_x3[:, 0, :], sq_x3[:, 1, :])
        x2_nat = singles.tile([128, 64], F32)
        nc.vector.tensor_add(x2_nat, tmp_x, sq_x3[:, 2, :])

        # Partition-crossing y2 scatters: group r gets only its quarter of
        # the row (y2t rows 16r..16r+15, j = c*128 + p), so the four DMAs
        # hit four different partitions in parallel.
        for r in range(4):
            nc.sync.dma_start(
                out=rhs_sb[32 * r + 3 : 32 * r + 4, r * 2048 : (r + 1) * 2048],
                in_=y2t[16 * r : 16 * r + 16, :],
            )

        for c in range(n_chunks):
            for q in range(QUADS):
                ps = ps_pool.tile([128, SUPER], F32, tag="ps")
                for r in range(4):
                    j0 = (r * 4 + q) * JTILE
                    nc.tensor.matmul(
                        out=ps[:, r * JTILE : (r + 1) * JTILE],
                        lhsT=lhsT_sb[32 * r : 32 * r + 4, c * P : (c + 1) * P],
                        rhs=rhs_sb[32 * r : 32 * r + 4, j0 : j0 + JTILE],
                        start=True,
                        stop=True,
                        tile_position=(32 * r, 0),
                    )
                # max over this superblock straight out of PSUM (1x path);
                # plain tensor_reduce avoids the per-op accumulator-readback
                # instruction and the full-width side write of ts+accum.
                nc.vector.tensor_reduce(
                    out=M_cols[:, c * QUADS + q : c * QUADS + q + 1],
                    in_=ps,
                    axis=mybir.AxisListType.X,
                    op=mybir.AluOpType.max,
                )

        # combine the per-superblock maxes: [128, (c q)] -> [128, c]
        nc.vector.tensor_reduce(
            out=M_nat,
            in_=M_cols.rearrange("p (c q) -> p c q", q=QUADS),
            axis=mybir.AxisListType.X,
            op=mybir.AluOpType.max,
        )

        # partial = sum_i (x2_i - 2*M_i) * SCALE ; then partition-sum via PE.
        M2 = singles.tile([128, n_chunks], F32)
        nc.vector.tensor_scalar_mul(M2, M_nat, -2.0)
        E_sum = singles.tile([128, n_chunks], F32)
        nc.vector.tensor_add(E_sum, x2_nat[:, 0:n_chunks], M2)
        part = singles.tile([128, 1], F32)
        nc.vector.tensor_scalar(
            out=scr[:, 0:n_chunks],
            in0=E_sum,
            scalar1=SCALE,
            scalar2=None,
            op0=mybir.AluOpType.mult,
            op1=mybir.AluOpType.add,
            accum_out=part,
        )
        ones_col = singles.tile([128, 1], F32)
        nc.vector.memset(ones_col, 1.0)
        ps_fin = ps_pool.tile([1, 1], F32, tag="ps")
        nc.tensor.matmul(
            out=ps_fin, lhsT=part, rhs=ones_col, start=True, stop=True
        )
        out_sb = singles.tile([1, 1], F32)
        nc.scalar.copy(out=out_sb, in_=ps_fin)
        nc.sync.dma_start(out=out[:], in_=out_sb)

    nc.compile()
    if not nc.is_finalized():
        nc.finalize()
    return nc


def make_in_maps(xyz1, xyz2):
    in_maps = []
    for b in range(B):
        x = np.ascontiguousarray(xyz1[b], dtype=np.float32)  # [8192, 3]
        y = np.ascontiguousarray(xyz2[b], dtype=np.float32)
        xT = np.empty((4, PTS), dtype=np.float32)
        xT[0:3] = x.T
        xT[3] = -0.5
        yT = np.empty((4, PTS), dtype=np.float32)
        yT[0:3] = y.T
        yT[3] = 0.0  # overwritten on device by y2
        y64 = np.ascontiguousarray(y.reshape(64, 384))
        x128 = np.ascontiguousarray(
            x.reshape(64, 128, 3).transpose(1, 0, 2).reshape(128, 192)
        )
        in_maps.append({"xT": xT, "yT": yT, "y64": y64, "x128": x128})
    return in_maps


def _run(xyz1, xyz2, trace=False):
    nc = build()
    in_maps = make_in_maps(xyz1, xyz2)
    res = run_bass_kernel_spmd(nc, in_maps, list(range(B)), trace=trace)
    total = np.float64(0.0)
    for r in res.results:
        total += np.float64(r["out"][0, 0])
    return np.asarray(total, dtype=np.float32), res


def kernel(xyz1, xyz2):
    out, _ = _run(np.asarray(xyz1), np.asarray(xyz2), trace=False)
    return out


# revision 18
# speedup vs baseline: 1.0888x; 1.0130x over previous
"""Chamfer distance (dist1 mean only) on 8 trn2 NeuronCores.

Sharding: data-parallel over batch B=8, one batch per core. Each core
computes sum_i min_j ||x_i - y_j||^2 / 65536 for its batch; host sums the
8 partial scalars.

Per-core algorithm:
  min_j d(i,j) = x2_i - 2 * max_j (x_i . y_j - 0.5*y2_j)
The inner term is a K=4 matmul: lhsT rows = (x0, x1, x2, -0.5),
rhs rows = (y0, y1, y2, y2), spread over the four PE row groups
(tile_position).  The max-reduction over j runs on VectorE as
tensor_scalar ops with a max accum_out, reading PSUM directly (the only
fast DVE path measured on this part); per-chunk partial maxes land in
M_cols and are combined with one small reduce at the end.
"""

from contextlib import ExitStack

import numpy as np

import concourse.bass as bass
import concourse.tile as tile
from concourse import bacc
from concourse import mybir
from concourse.bass_utils import run_bass_kernel_spmd

F32 = mybir.dt.float32

B = 8
PTS = 8192            # points per batch (both clouds)
P = 128               # i-chunk size (PSUM partitions)
JTILE = 512           # matmul free dim (one PSUM bank)
SUPER = 2048          # superblock free dim (4 banks)
QUADS = PTS // SUPER  # 4 superblocks per i-chunk
NEG_INIT = -3.0e38
SCALE = 1.0 / (B * PTS)  # each core contributes sum/65536


def build(n_chunks=PTS // P):
    nc = bacc.Bacc(None)
    xT = nc.declare_dram_parameter("xT", [4, PTS], F32, isOutput=False)
    yT = nc.declare_dram_parameter("yT", [4, PTS], F32, isOutput=False)
    y64 = nc.declare_dram_parameter("y64", [64, 384], F32, isOutput=False)
    x128 = nc.declare_dram_parameter("x128", [128, 192], F32, isOutput=False)
    out = nc.declare_dram_parameter("out", [1, 1], F32, isOutput=True)

    with ExitStack() as ctx:
        tc = ctx.enter_context(tile.TileContext(nc))
        singles = ctx.enter_context(tc.tile_pool(name="singles", bufs=1))
        ps_pool = ctx.enter_context(tc.tile_pool(name="ps", bufs=2, space="PSUM"))

        lhsT_sb = singles.tile([128, PTS], F32)
        rhs_sb = singles.tile([128, PTS], F32)
        scr = singles.tile([128, SUPER], F32)
        M_cols = singles.tile([128, QUADS * n_chunks], F32)
        M_nat = singles.tile([128, n_chunks], F32)

        # x data replicated into the four 32-partition row groups.  Row
        # group r only ever consumes the contiguous j-range
        # [r*2048, (r+1)*2048) (see the main loop), so its y rows are
        # loaded for that quarter only.
        # xT is loaded in column quarters so early chunks (which read only
        # the first lhsT columns) can start before the whole fill finishes.
        for h in range(4):
            hsl = slice(h * 2048, (h + 1) * 2048)
            for r in range(4):
                nc.sync.dma_start(out=lhsT_sb[32 * r : 32 * r + 4, hsl], in_=xT[:, hsl])
        for r in range(4):
            nc.sync.dma_start(
                out=rhs_sb[32 * r : 32 * r + 3, r * 2048 : (r + 1) * 2048],
                in_=yT[0:3, r * 2048 : (r + 1) * 2048],
            )

        # y2[j] = |y_j|^2 computed in [64,128] layout, then flattened into the
        # j-ordered row (j = c*128 + p ordering matches yT columns).
        y64_sb = singles.tile([64, 384], F32)
        nc.scalar.dma_start(out=y64_sb, in_=y64[:])
        sq_y = singles.tile([64, 384], F32)
        nc.vector.tensor_mul(sq_y, y64_sb, y64_sb)
        sq_y3 = sq_y.rearrange("p (q d) -> p d q", d=3)
        tmp_y = singles.tile([64, 128], F32)
        nc.vector.tensor_add(tmp_y, sq_y3[:, 0, :], sq_y3[:, 1, :])
        y2t = singles.tile([64, 128], F32)
        nc.vector.tensor_add(y2t, tmp_y, sq_y3[:, 2, :])

        # x2[i] = |x_i|^2 in [128, n_chunks] layout (i = c*128 + p).
        x128_sb = singles.tile([128, 192], F32)
        nc.scalar.dma_start(out=x128_sb, in_=x128[:])
        sq_x = singles.tile([128, 192], F32)
        nc.vector.tensor_mul(sq_x, x128_sb, x128_sb)
        sq_x3 = sq_x.rearrange("p (q d) -> p d q", d=3)
        tmp_x = singles.tile([128, 64], F32)
        nc.vector.tensor_add(tmp_x, sq_x3[:, 0, :], sq_x3[:, 1, :])
        x2_nat = singles.tile([128, 64], F32)
        nc.vector.tensor_add(x2_nat, tmp_x, sq_x3[:, 2, :])

        # Partition-crossing y2 scatters: group r gets only its quarter of
        # the row (y2t rows 16r..16r+15, j = c*128 + p), so the four DMAs
        # hit four different partitions in parallel.
        for r in range(4):
            nc.sync.dma_start(
                out=rhs_sb[32 * r + 3 : 32 * r + 4, r * 2048 : (r + 1) * 2048],
                in_=y2t[16 * r : 16 * r + 16, :],
            )

        for c in range(n_chunks):
            for q in range(QUADS):
                ps = ps_pool.tile([128, SUPER], F32, tag="ps")
                for r in range(4):
                    j0 = (r * 4 + q) * JTILE
                    nc.tensor.matmul(
                        out=ps[:, r * JTILE : (r + 1) * JTILE],
                        lhsT=lhsT_sb[32 * r : 32 * r + 4, c * P : (c + 1) * P],
                        rhs=rhs_sb[32 * r : 32 * r + 4, j0 : j0 + JTILE],
                        start=True,
                        stop=True,
                        tile_position=(32 * r, 0),
                    )
                # max over this superblock straight out of PSUM (1x path);
                # plain tensor_reduce avoids the per-op accumulator-readback
                # instruction and the full-width side write of ts+accum.
                nc.vector.tensor_reduce(
                    out=M_cols[:, c * QUADS + q : c * QUADS + q + 1],
                    in_=ps,
                    axis=mybir.AxisListType.X,
                    op=mybir.AluOpType.max,
                )

        # combine the per-superblock maxes: [128, (c q)] -> [128, c]
        nc.vector.tensor_reduce(
            out=M_nat,
            in_=M_cols.rearrange("p (c q) -> p c q", q=QUADS),
            axis=mybir.AxisListType.X,
            op=mybir.AluOpType.max,
        )

        # partial = sum_i (x2_i - 2*M_i) * SCALE ; then partition-sum via PE.
        M2 = singles.tile([128, n_chunks], F32)
        nc.vector.tensor_scalar_mul(M2, M_nat, -2.0)
        E_sum = singles.tile([128, n_chunks], F32)
        nc.vector.tensor_add(E_sum, x2_nat[:, 0:n_chunks], M2)
        part = singles.tile([128, 1], F32)
        nc.vector.tensor_scalar(
            out=scr[:, 0:n_chunks],
            in0=E_sum,
            scalar1=SCALE,
            scalar2=None,
            op0=mybir.AluOpType.mult,
            op1=mybir.AluOpType.add,
            accum_out=part,
        )
        ones_col = singles.tile([128, 1], F32)
        nc.vector.memset(ones_col, 1.0)
        ps_fin = ps_pool.tile([1, 1], F32, tag="ps")
        nc.tensor.matmul(
            out=ps_fin, lhsT=part, rhs=ones_col, start=True, stop=True
        )
        out_sb = singles.tile([1, 1], F32)
        nc.scalar.copy(out=out_sb, in_=ps_fin)
        nc.sync.dma_start(out=out[:], in_=out_sb)

    nc.compile()
    if not nc.is_finalized():
        nc.finalize()
    return nc


def make_in_maps(xyz1, xyz2):
    in_maps = []
    for b in range(B):
        x = np.ascontiguousarray(xyz1[b], dtype=np.float32)  # [8192, 3]
        y = np.ascontiguousarray(xyz2[b], dtype=np.float32)
        xT = np.empty((4, PTS), dtype=np.float32)
        xT[0:3] = x.T
        xT[3] = -0.5
        yT = np.empty((4, PTS), dtype=np.float32)
        yT[0:3] = y.T
        yT[3] = 0.0  # overwritten on device by y2
        y64 = np.ascontiguousarray(y.reshape(64, 384))
        x128 = np.ascontiguousarray(
            x.reshape(64, 128, 3).transpose(1, 0, 2).reshape(128, 192)
        )
        in_maps.append({"xT": xT, "yT": yT, "y64": y64, "x128": x128})
    return in_maps


def _run(xyz1, xyz2, trace=False):
    nc = build()
    in_maps = make_in_maps(xyz1, xyz2)
    res = run_bass_kernel_spmd(nc, in_maps, list(range(B)), trace=trace)
    total = np.float64(0.0)
    for r in res.results:
        total += np.float64(r["out"][0, 0])
    return np.asarray(total, dtype=np.float32), res


def kernel(xyz1, xyz2):
    out, _ = _run(np.asarray(xyz1), np.asarray(xyz2), trace=False)
    return out


# revision 19
# speedup vs baseline: 1.1014x; 1.0116x over previous
"""Chamfer distance (dist1 mean only) on 8 trn2 NeuronCores.

Sharding: data-parallel over batch B=8, one batch per core. Each core
computes sum_i min_j ||x_i - y_j||^2 / 65536 for its batch; host sums the
8 partial scalars.

Per-core algorithm:
  min_j d(i,j) = x2_i - 2 * max_j (x_i . y_j - 0.5*y2_j)
The inner term is a K=4 matmul: lhsT rows = (x0, x1, x2, -0.5),
rhs rows = (y0, y1, y2, y2), spread over the four PE row groups
(tile_position).  The max-reduction over j runs on VectorE as
tensor_scalar ops with a max accum_out, reading PSUM directly (the only
fast DVE path measured on this part); per-chunk partial maxes land in
M_cols and are combined with one small reduce at the end.
"""

from contextlib import ExitStack

import numpy as np

import concourse.bass as bass
import concourse.tile as tile
from concourse import bacc
from concourse import mybir
from concourse.bass_utils import run_bass_kernel_spmd

F32 = mybir.dt.float32

B = 8
PTS = 8192            # points per batch (both clouds)
P = 128               # i-chunk size (PSUM partitions)
JTILE = 512           # matmul free dim (one PSUM bank)
SUPER = 2048          # superblock free dim (4 banks)
QUADS = PTS // SUPER  # 4 superblocks per i-chunk
NEG_INIT = -3.0e38
SCALE = 1.0 / (B * PTS)  # each core contributes sum/65536


def build(n_chunks=PTS // P):
    nc = bacc.Bacc(None)
    xT = nc.declare_dram_parameter("xT", [4, PTS], F32, isOutput=False)
    yT = nc.declare_dram_parameter("yT", [4, PTS], F32, isOutput=False)
    y64 = nc.declare_dram_parameter("y64", [64, 384], F32, isOutput=False)
    x128 = nc.declare_dram_parameter("x128", [128, 192], F32, isOutput=False)
    out = nc.declare_dram_parameter("out", [1, 1], F32, isOutput=True)

    with ExitStack() as ctx:
        tc = ctx.enter_context(tile.TileContext(nc))
        singles = ctx.enter_context(tc.tile_pool(name="singles", bufs=1))
        ps_pool = ctx.enter_context(tc.tile_pool(name="ps", bufs=2, space="PSUM"))

        lhsT_sb = singles.tile([128, PTS], F32)
        rhs_sb = singles.tile([128, PTS], F32)
        scr = singles.tile([128, SUPER], F32)
        M_cols = singles.tile([128, QUADS * n_chunks], F32)
        M_nat = singles.tile([128, n_chunks], F32)

        # x data replicated into the four 32-partition row groups.  Row
        # group r only ever consumes the contiguous j-range
        # [r*2048, (r+1)*2048) (see the main loop), so its y rows are
        # loaded for that quarter only.
        # Issue order = first-superblock critical path: the y quarters and
        # xT column-quarter 0 go first; the remaining xT quarters are issued
        # last and overlap with the running main loop (chunk c only reads
        # lhsT columns c*128..c*128+127).
        for r in range(4):
            nc.sync.dma_start(
                out=rhs_sb[32 * r : 32 * r + 3, r * 2048 : (r + 1) * 2048],
                in_=yT[0:3, r * 2048 : (r + 1) * 2048],
            )
        for r in range(4):
            nc.sync.dma_start(out=lhsT_sb[32 * r : 32 * r + 4, 0:2048], in_=xT[:, 0:2048])

        # y2[j] = |y_j|^2 computed in [64,128] layout, then flattened into the
        # j-ordered row (j = c*128 + p ordering matches yT columns).
        y64_sb = singles.tile([64, 384], F32)
        nc.scalar.dma_start(out=y64_sb, in_=y64[:])
        sq_y = singles.tile([64, 384], F32)
        nc.vector.tensor_mul(sq_y, y64_sb, y64_sb)
        sq_y3 = sq_y.rearrange("p (q d) -> p d q", d=3)
        tmp_y = singles.tile([64, 128], F32)
        nc.vector.tensor_add(tmp_y, sq_y3[:, 0, :], sq_y3[:, 1, :])
        y2t = singles.tile([64, 128], F32)
        nc.vector.tensor_add(y2t, tmp_y, sq_y3[:, 2, :])

        # x2[i] = |x_i|^2 in [128, n_chunks] layout (i = c*128 + p).
        x128_sb = singles.tile([128, 192], F32)
        nc.scalar.dma_start(out=x128_sb, in_=x128[:])
        sq_x = singles.tile([128, 192], F32)
        nc.vector.tensor_mul(sq_x, x128_sb, x128_sb)
        sq_x3 = sq_x.rearrange("p (q d) -> p d q", d=3)
        tmp_x = singles.tile([128, 64], F32)
        nc.vector.tensor_add(tmp_x, sq_x3[:, 0, :], sq_x3[:, 1, :])
        x2_nat = singles.tile([128, 64], F32)
        nc.vector.tensor_add(x2_nat, tmp_x, sq_x3[:, 2, :])

        # Partition-crossing y2 scatters: group r gets only its quarter of
        # the row (y2t rows 16r..16r+15, j = c*128 + p), so the four DMAs
        # hit four different partitions in parallel.
        for r in range(4):
            nc.sync.dma_start(
                out=rhs_sb[32 * r + 3 : 32 * r + 4, r * 2048 : (r + 1) * 2048],
                in_=y2t[16 * r : 16 * r + 16, :],
            )

        # Remaining xT quarters: needed only from chunk 16 onward.
        for h in range(1, 4):
            hsl = slice(h * 2048, (h + 1) * 2048)
            for r in range(4):
                nc.sync.dma_start(out=lhsT_sb[32 * r : 32 * r + 4, hsl], in_=xT[:, hsl])

        for c in range(n_chunks):
            for q in range(QUADS):
                ps = ps_pool.tile([128, SUPER], F32, tag="ps")
                for r in range(4):
                    j0 = (r * 4 + q) * JTILE
                    nc.tensor.matmul(
                        out=ps[:, r * JTILE : (r + 1) * JTILE],
                        lhsT=lhsT_sb[32 * r : 32 * r + 4, c * P : (c + 1) * P],
                        rhs=rhs_sb[32 * r : 32 * r + 4, j0 : j0 + JTILE],
                        start=True,
                        stop=True,
                        tile_position=(32 * r, 0),
                    )
                # max over this superblock straight out of PSUM (1x path);
                # plain tensor_reduce avoids the per-op accumulator-readback
                # instruction and the full-width side write of ts+accum.
                nc.vector.tensor_reduce(
                    out=M_cols[:, c * QUADS + q : c * QUADS + q + 1],
                    in_=ps,
                    axis=mybir.AxisListType.X,
                    op=mybir.AluOpType.max,
                )

        # combine the per-superblock maxes: [128, (c q)] -> [128, c]
        nc.vector.tensor_reduce(
            out=M_nat,
            in_=M_cols.rearrange("p (c q) -> p c q", q=QUADS),
            axis=mybir.AxisListType.X,
            op=mybir.AluOpType.max,
        )

        # partial = sum_i (x2_i - 2*M_i) * SCALE ; then partition-sum via PE.
        M2 = singles.tile([128, n_chunks], F32)
        nc.vector.tensor_scalar_mul(M2, M_nat, -2.0)
        E_sum = singles.tile([128, n_chunks], F32)
        nc.vector.tensor_add(E_sum, x2_nat[:, 0:n_chunks], M2)
        part = singles.tile([128, 1], F32)
        nc.vector.tensor_scalar(
            out=scr[:, 0:n_chunks],
            in0=E_sum,
            scalar1=SCALE,
            scalar2=None,
            op0=mybir.AluOpType.mult,
            op1=mybir.AluOpType.add,
            accum_out=part,
        )
        ones_col = singles.tile([128, 1], F32)
        nc.vector.memset(ones_col, 1.0)
        ps_fin = ps_pool.tile([1, 1], F32, tag="ps")
        nc.tensor.matmul(
            out=ps_fin, lhsT=part, rhs=ones_col, start=True, stop=True
        )
        out_sb = singles.tile([1, 1], F32)
        nc.scalar.copy(out=out_sb, in_=ps_fin)
        nc.sync.dma_start(out=out[:], in_=out_sb)

    nc.compile()
    if not nc.is_finalized():
        nc.finalize()
    return nc


def make_in_maps(xyz1, xyz2):
    in_maps = []
    for b in range(B):
        x = np.ascontiguousarray(xyz1[b], dtype=np.float32)  # [8192, 3]
        y = np.ascontiguousarray(xyz2[b], dtype=np.float32)
        xT = np.empty((4, PTS), dtype=np.float32)
        xT[0:3] = x.T
        xT[3] = -0.5
        yT = np.empty((4, PTS), dtype=np.float32)
        yT[0:3] = y.T
        yT[3] = 0.0  # overwritten on device by y2
        y64 = np.ascontiguousarray(y.reshape(64, 384))
        x128 = np.ascontiguousarray(
            x.reshape(64, 128, 3).transpose(1, 0, 2).reshape(128, 192)
        )
        in_maps.append({"xT": xT, "yT": yT, "y64": y64, "x128": x128})
    return in_maps


def _run(xyz1, xyz2, trace=False):
    nc = build()
    in_maps = make_in_maps(xyz1, xyz2)
    res = run_bass_kernel_spmd(nc, in_maps, list(range(B)), trace=trace)
    total = np.float64(0.0)
    for r in res.results:
        total += np.float64(r["out"][0, 0])
    return np.asarray(total, dtype=np.float32), res


def kernel(xyz1, xyz2):
    out, _ = _run(np.asarray(xyz1), np.asarray(xyz2), trace=False)
    return out
